# revision 4
# baseline (speedup 1.0000x reference)
"""GPT2 attention block (LN -> QKV -> causal attention over past KV -> proj)
on 8 Trainium2 NeuronCores, batch-parallel (one batch element per core).

Key structural facts exploited:
  * With q_len=1024, WINDOW=1024 and the reference's "no past-length offset"
    sliding-window mask, keep = (j <= i) for i in [0,1024) -- so only the
    first 1024 key positions (== past_k/past_v entirely) are ever attended.
    The freshly projected k/v never enter attention; they are only emitted
    as outputs (concat on host).
  * Everything device-side runs in transposed orientation (feature/key on
    partitions, sequence on the free dim) so LayerNorm statistics and
    softmax denominators become TensorEngine ones-matmuls; no on-device
    transposes and no partition-dim reductions are needed.
"""

import os
import sys

sys.path.insert(0, "/opt/trn_rl_repo")

import ml_dtypes
import numpy as np

import concourse.bass as bass  # noqa: F401  (import keeps bass registered)
import concourse.tile as tile
from concourse import bacc, mybir
from concourse.bass_utils import run_bass_kernel_spmd

F32 = mybir.dt.float32
F32R = mybir.dt.float32r
BF16 = mybir.dt.bfloat16
BF16_NP = ml_dtypes.bfloat16

S, D, H, DH = 1024, 768, 12, 64
FT = D // 128  # 6 feature tiles of 128
CH = 3 * D // 128  # 18 qkv column chunks of 128
NKJ = S // 128  # 8 key chunks of 128
SPANQ = 256  # query span per attention step
NSI = S // SPANQ  # 4 spans
EPS = 1e-5
AF = mybir.ActivationFunctionType
N_CORES = 8


def _r(ap):
    """fp32 -> fp32r view: full-rate PE matmul for moving dims >= 256."""
    return ap.bitcast(F32R)


def _emit(nc, tc, xT, kTd, vAd, wqd, wpd, bqd, bpd, mAd, mBd, outd, kvd):
    ctxs = []

    def pool(**kw):
        p = tc.tile_pool(**kw)
        ctxs.append(p)
        return p.__enter__()

    resA = pool(name="resA", bufs=1)  # xT slot, later reused by mergedT
    res = pool(name="res", bufs=1)  # long-lived tensors
    wqp = pool(name="wqs", bufs=2)  # streamed qkv-weight thirds
    lnt = pool(name="lnt", bufs=1)  # layernorm temporaries
    sqp = pool(name="sqp", bufs=2)  # x^2 half-span tiles
    prp = pool(name="prp", bufs=2)  # per-(head,span) probabilities
    kvop = pool(name="kvop", bufs=2)  # kv eviction staging
    oop = pool(name="oop", bufs=2)  # out eviction staging
    atsp = pool(name="atsp", bufs=2)  # reciprocal + broadcast staging
    psb = pool(name="psb", bufs=3, space="PSUM")  # matmul accumulators
    psm = pool(name="psm", bufs=5, space="PSUM")  # stats/attn/broadcast

    # ---- resident loads -------------------------------------------------
    xT_sb = resA.tile([128, FT, S], F32, tag="big", name="xT_sb")
    nc.sync.dma_start(out=xT_sb, in_=xT.rearrange("(t p) s -> p t s", p=128))

    kT_sb = res.tile([128, FT * S], BF16, name="kT_sb")
    nc.sync.dma_start(out=kT_sb, in_=kTd)
    vA_sb = res.tile([128, H * NKJ * (DH + 1)], BF16, name="vA_sb")
    nc.sync.dma_start(out=vA_sb, in_=vAd)
    wp_sb = res.tile([128, FT, D], BF16, name="wp_sb")
    nc.sync.dma_start(out=wp_sb, in_=wpd.rearrange("(t p) e -> p t e", p=128))
    bq_sb = res.tile([128, CH], F32, name="bq_sb")
    nc.sync.dma_start(out=bq_sb, in_=bqd.rearrange("a b -> b a"))
    bp_sb = res.tile([1, D], BF16, name="bp_sb")
    nc.sync.dma_start(out=bp_sb, in_=bpd)
    mA_sb = res.tile([128, SPANQ], BF16, name="mA_sb")
    nc.sync.dma_start(out=mA_sb, in_=mAd)
    mB_sb = res.tile([128, SPANQ], BF16, name="mB_sb")
    nc.sync.dma_start(out=mB_sb, in_=mBd)

    onesP = res.tile([128, 1], F32, name="onesP")
    nc.vector.memset(onesP, 1.0)
    ones1 = res.tile([1, 128], F32, name="ones1")
    nc.vector.memset(ones1, 1.0)
    ones1_bf = res.tile([1, 128], BF16, name="ones1_bf")
    nc.vector.memset(ones1_bf, 1.0)

    x1T_sb = res.tile([128, FT, S], BF16, name="x1T_sb")
    qT_sb = res.tile([128, FT, S], BF16, name="qT_sb")

    # ---- phase 1: LayerNorm statistics via ones-matmuls -----------------
    # sum_x[q] and sum_x2[q] accumulate over all 6 feature tiles.
    mu_sb = lnt.tile([1, S], F32, tag="mu", name="mu_sb")
    msq_sb = lnt.tile([1, S], F32, tag="msq", name="msq_sb")
    for sp in range(2):
        ssl = slice(sp * 512, (sp + 1) * 512)
        sum_ps = psm.tile([1, 512], F32, tag="m", name="sum_ps")
        sq_ps = psm.tile([1, 512], F32, tag="m", name="sq_ps")
        for t in range(FT):
            nc.tensor.matmul(
                sum_ps,
                onesP,
                xT_sb[:, t, ssl],
                start=(t == 0),
                stop=(t == FT - 1),
            )
            sq_t = sqp.tile([128, 512], F32, tag="sq", name="sq_t")
            nc.scalar.activation(sq_t, xT_sb[:, t, ssl], AF.Square)
            nc.tensor.matmul(
                sq_ps, onesP, sq_t, start=(t == 0), stop=(t == FT - 1)
            )
        nc.scalar.mul(mu_sb[:, ssl], sum_ps, 1.0 / D)
        nc.scalar.mul(msq_sb[:, ssl], sq_ps, 1.0 / D)

    tmp_sb = lnt.tile([1, S], F32, tag="tmp", name="tmp_sb")
    nc.vector.tensor_mul(tmp_sb, mu_sb, mu_sb)
    nc.vector.tensor_sub(msq_sb, msq_sb, tmp_sb)  # msq now = var
    eps_sb = lnt.tile([1, 1], F32, tag="eps", name="eps_sb")
    nc.vector.memset(eps_sb, EPS)
    sd_sb = lnt.tile([1, S], F32, tag="sd", name="sd_sb")
    nc.scalar.activation(sd_sb, msq_sb, AF.Sqrt, bias=eps_sb)
    rs_sb = lnt.tile([1, S], F32, tag="rs", name="rs_sb")
    nc.vector.reciprocal(rs_sb, sd_sb)
    nmr_sb = lnt.tile([1, S], F32, tag="nmr", name="nmr_sb")
    nc.vector.tensor_mul(nmr_sb, mu_sb, rs_sb)

    # broadcast rs and mu*rs across 128 partitions with K=1 matmuls
    rs_bc = lnt.tile([128, S], F32, tag="rsbc", name="rs_bc")
    nmr_bc = lnt.tile([128, S], F32, tag="nmrbc", name="nmr_bc")
    for sp in range(2):
        ssl = slice(sp * 512, (sp + 1) * 512)
        bc_ps = psb.tile([128, 512], F32, tag="mm", name="bc_ps")
        nc.tensor.matmul(bc_ps, ones1, rs_sb[:, ssl])
        nc.scalar.copy(rs_bc[:, ssl], bc_ps)
        bc2_ps = psb.tile([128, 512], F32, tag="mm", name="bc2_ps")
        nc.tensor.matmul(bc2_ps, ones1, nmr_sb[:, ssl])
        nc.scalar.copy(nmr_bc[:, ssl], bc2_ps)

    # x1 = x*rs - mu*rs  (normalized; ln affine folded into wq/bq on host)
    for t in range(FT):
        x1tmp = sqp.tile([128, S], F32, tag="x1tmp", name="x1tmp")
        nc.vector.tensor_mul(x1tmp, xT_sb[:, t, :], rs_bc)
        nc.vector.tensor_sub(x1T_sb[:, t, :], x1tmp, nmr_bc)

    # ---- phase 2: QKV projection ---------------------------------------
    # qkvT[c, q] = sum_f wq[f, c] * x1T[f, q]; evict with bias on ScalarE.
    for g in range(3):  # stream wq in thirds of 768 columns
        wq_sb = wqp.tile([128, FT, D], BF16, tag="wq", name="wq_sb")
        nc.sync.dma_start(
            out=wq_sb,
            in_=wqd[:, g * D : (g + 1) * D].rearrange("(t p) c -> p t c", p=128),
        )
        for ccl in range(FT):
            cc = g * FT + ccl
            if cc >= FT:
                kv_sb = kvop.tile([128, S], F32, tag="kv", name="kv_sb")
            for sp in range(2):
                ssl = slice(sp * 512, (sp + 1) * 512)
                qkv_ps = psb.tile([128, 512], F32, tag="mm", name="qkv_ps")
                for t in range(FT):
                    nc.tensor.matmul(
                        qkv_ps,
                        wq_sb[:, t, ccl * 128 : (ccl + 1) * 128],
                        x1T_sb[:, t, ssl],
                        start=(t == 0),
                        stop=(t == FT - 1),
                    )
                if cc < FT:  # q columns -> resident bf16 qT
                    nc.scalar.activation(
                        qT_sb[:, cc, ssl],
                        qkv_ps,
                        AF.Identity,
                        bias=bq_sb[:, cc : cc + 1],
                    )
                else:  # k/v columns -> fp32 out through HBM
                    nc.scalar.activation(
                        kv_sb[:, ssl],
                        qkv_ps,
                        AF.Identity,
                        bias=bq_sb[:, cc : cc + 1],
                    )
            if cc >= FT:
                nc.sync.dma_start(
                    out=kvd[(cc - FT) * 128 : (cc - FT + 1) * 128, :], in_=kv_sb
                )

    # ---- phase 3: attention over past K/V ------------------------------
    mergedT_sb = resA.tile([128, FT, S], BF16, tag="big", name="mergedT_sb")
    for t in range(FT):
        for o in range(2):
            h = 2 * t + o
            orow = slice(o * 64, (o + 1) * 64)
            for si in range(NSI):
                qsl = slice(si * SPANQ, (si + 1) * SPANQ)
                nblk = 2 * si + 2
                probs_sb = prp.tile(
                    [128, NKJ, SPANQ], BF16, tag="probs", name="probs_sb"
                )
                for kj in range(nblk):
                    sc_ps = psb.tile([128, SPANQ], F32, tag="mm", name="sc_ps")
                    nc.tensor.matmul(
                        sc_ps,
                        kT_sb[orow, t * S + kj * 128 : t * S + (kj + 1) * 128],
                        qT_sb[orow, t, qsl],
                    )
                    nc.scalar.activation(
                        probs_sb[:, kj, :], sc_ps, AF.Exp, scale=0.125
                    )
                    if kj == 2 * si:
                        nc.vector.tensor_mul(
                            probs_sb[:, kj, :], probs_sb[:, kj, :], mA_sb
                        )
                    elif kj == 2 * si + 1:
                        nc.vector.tensor_mul(
                            probs_sb[:, kj, :], probs_sb[:, kj, :], mB_sb
                        )
                at_ps = psm.tile([DH + 1, SPANQ], F32, tag="m", name="at_ps")
                for kj in range(nblk):
                    nc.tensor.matmul(
                        at_ps,
                        vA_sb[:, (h * NKJ + kj) * (DH + 1) : (h * NKJ + kj + 1) * (DH + 1)],
                        probs_sb[:, kj, :],
                        start=(kj == 0),
                        stop=(kj == nblk - 1),
                    )
                rec_sb = atsp.tile([1, SPANQ], BF16, tag="rec", name="rec_sb")
                nc.vector.reciprocal(rec_sb, at_ps[DH : DH + 1, :])
                bcr_ps = psm.tile([DH, SPANQ], F32, tag="m", name="bcr_ps")
                nc.tensor.matmul(bcr_ps, ones1_bf[:, :DH], rec_sb)
                rbc_sb = atsp.tile([DH, SPANQ], F32, tag="rbc", name="rbc_sb")
                nc.scalar.copy(rbc_sb, bcr_ps)
                nc.vector.tensor_mul(
                    mergedT_sb[orow, t, qsl], at_ps[0:DH, :], rbc_sb
                )

    # ---- phase 4: output projection ------------------------------------
    for qc in range(8):
        for ep in range(2):
            esl = slice(ep * 384, (ep + 1) * 384)
            o_ps = psb.tile([128, 384], F32, tag="mm", name="o_ps")
            for t in range(FT):
                nc.tensor.matmul(
                    o_ps,
                    mergedT_sb[:, t, qc * 128 : (qc + 1) * 128],
                    wp_sb[:, t, esl],
                    start=(t == 0),
                    stop=False,
                )
            nc.tensor.matmul(
                o_ps, ones1_bf, bp_sb[:, esl], start=False, stop=True
            )
            o_sb = oop.tile([128, 384], F32, tag="oo", name="o_sb")
            nc.scalar.copy(o_sb, o_ps)
            nc.sync.dma_start(out=outd[qc * 128 : (qc + 1) * 128, esl], in_=o_sb)

    for p in reversed(ctxs):
        p.__exit__(None, None, None)


def _build_program():
    nc = bacc.Bacc(
        "TRN2", target_bir_lowering=False, debug=False, num_devices=N_CORES
    )
    xT = nc.dram_tensor("xT", [D, S], F32, kind="ExternalInput").ap()
    kTd = nc.dram_tensor("kTd", [128, FT * S], BF16, kind="ExternalInput").ap()
    vAd = nc.dram_tensor(
        "vAd", [128, H * NKJ * (DH + 1)], BF16, kind="ExternalInput"
    ).ap()
    wqd = nc.dram_tensor("wqd", [D, 3 * D], BF16, kind="ExternalInput").ap()
    wpd = nc.dram_tensor("wpd", [D, D], BF16, kind="ExternalInput").ap()
    bqd = nc.dram_tensor("bqd", [CH, 128], F32, kind="ExternalInput").ap()
    bpd = nc.dram_tensor("bpd", [1, D], BF16, kind="ExternalInput").ap()
    mAd = nc.dram_tensor("mAd", [128, SPANQ], BF16, kind="ExternalInput").ap()
    mBd = nc.dram_tensor("mBd", [128, SPANQ], BF16, kind="ExternalInput").ap()
    outd = nc.dram_tensor("outd", [S, D], F32, kind="ExternalOutput").ap()
    kvd = nc.dram_tensor("kvd", [2 * D, S], F32, kind="ExternalOutput").ap()

    with tile.TileContext(nc) as tc:
        with nc.allow_low_precision(reason="bf16 matmul operands; fp32 PSUM accumulation throughout"):
            _emit(nc, tc, xT, kTd, vAd, wqd, wpd, bqd, bpd, mAd, mBd, outd, kvd)
    nc.compile()
    return nc


_CACHE = {}
LAST_RUN = {}


def _get_nc():
    if "nc" not in _CACHE:
        _CACHE["nc"] = _build_program()
    return _CACHE["nc"]


def _host_prep(inputs):
    x = np.asarray(inputs["x"], np.float32)
    past_k = np.asarray(inputs["past_k"], np.float32)
    past_v = np.asarray(inputs["past_v"], np.float32)
    ln_w = np.asarray(inputs["ln_w"], np.float32)
    ln_b = np.asarray(inputs["ln_b"], np.float32)
    caw = np.asarray(inputs["c_attn_w"], np.float32)
    cab = np.asarray(inputs["c_attn_b"], np.float32)
    cpw = np.asarray(inputs["c_proj_w"], np.float32)
    cpb = np.asarray(inputs["c_proj_b"], np.float32)

    wq = (ln_w[:, None] * caw).astype(BF16_NP)
    bq = np.ascontiguousarray((cab + ln_b @ caw).reshape(CH, 128), np.float32)
    wp = cpw.astype(BF16_NP)
    bp = np.ascontiguousarray(cpb.reshape(1, D)).astype(BF16_NP)
    kk = np.arange(128)[:, None]
    qq = np.arange(SPANQ)[None, :]
    mA = (kk <= qq).astype(BF16_NP)
    mB = ((kk + 128) <= qq).astype(BF16_NP)

    in_maps = []
    for b in range(N_CORES):
        xT = np.ascontiguousarray(x[b].T)  # [768, 1024]
        kT = np.ascontiguousarray(
            past_k[b].transpose(0, 2, 1).reshape(FT, 128, S).transpose(1, 0, 2)
            .reshape(128, FT * S)
        ).astype(BF16_NP)
        pv = past_v[b].reshape(H, NKJ, 128, DH).transpose(2, 0, 1, 3)
        vA = np.concatenate(
            [pv, np.ones((128, H, NKJ, 1), np.float32)], axis=3
        ).reshape(128, H * NKJ * (DH + 1)).astype(BF16_NP)
        in_maps.append(
            {
                "xT": xT,
                "kTd": kT,
                "vAd": vA,
                "wqd": wq,
                "wpd": wp,
                "bqd": bq,
                "bpd": bp,
                "mAd": mA,
                "mBd": mB,
            }
        )
    return in_maps, past_k, past_v


def kernel(**inputs):
    in_maps, past_k, past_v = _host_prep(inputs)
    nc = _get_nc()
    trace = bool(os.environ.get("BASS_KERNEL_TRACE"))
    res = run_bass_kernel_spmd(
        nc, in_maps, list(range(N_CORES)), trace=trace
    )
    LAST_RUN["exec_time_ns"] = res.exec_time_ns
    LAST_RUN["mean_exec_time_ns"] = res.mean_exec_time_ns
    LAST_RUN["profile_json"] = res.profile_json

    B = past_k.shape[0]
    out = np.empty((B, S, D), np.float32)
    k_new = np.empty((B, H, S, DH), np.float32)
    v_new = np.empty((B, H, S, DH), np.float32)
    for b in range(B):
        r = res.results[b]
        out[b] = r["outd"]
        kv = r["kvd"]  # [1536, 1024]
        k_new[b] = kv[:D].reshape(H, DH, S).transpose(0, 2, 1)
        v_new[b] = kv[D:].reshape(H, DH, S).transpose(0, 2, 1)
    k_out = np.concatenate([past_k, k_new], axis=2)
    v_out = np.concatenate([past_v, v_new], axis=2)
    return out, k_out, v_out


if __name__ == "__main__":
    # CoreSim smoke test on a single core (batch 0) against numpy.
    from concourse.bass_interp import CoreSim

    rng = np.random.default_rng(0)
    B = N_CORES
    inputs = {
        "x": rng.standard_normal((B, S, D), np.float32),
        "attn_mask": np.ones((B, 2 * S), bool),
        "past_k": rng.standard_normal((B, H, S, DH), np.float32),
        "past_v": rng.standard_normal((B, H, S, DH), np.float32),
        "ln_w": 1.0 + 0.1 * rng.standard_normal(D).astype(np.float32),
        "ln_b": 0.1 * rng.standard_normal(D).astype(np.float32),
        "c_attn_w": (0.02 * rng.standard_normal((D, 3 * D))).astype(np.float32),
        "c_attn_b": 0.01 * rng.standard_normal(3 * D).astype(np.float32),
        "c_proj_w": (0.02 * rng.standard_normal((D, D))).astype(np.float32),
        "c_proj_b": 0.01 * rng.standard_normal(D).astype(np.float32),
    }
    in_maps, past_k, past_v = _host_prep(inputs)
    nc = _build_program()
    print("program built; instructions:", sum(1 for _ in nc.m.instructions)
          if hasattr(nc.m, "instructions") else "?")
    sim = CoreSim(nc)
    for name, val in in_maps[0].items():
        sim.tensor(name)[:] = val
    sim.simulate()
    out_dev = np.array(sim.tensor("outd"))
    kv_dev = np.array(sim.tensor("kvd"))

    # numpy reference for batch 0
    x = inputs["x"][0]
    mu = x.mean(-1, keepdims=True)
    var = ((x - mu) ** 2).mean(-1, keepdims=True)
    x1 = (x - mu) / np.sqrt(var + EPS) * inputs["ln_w"] + inputs["ln_b"]
    qkv = x1 @ inputs["c_attn_w"] + inputs["c_attn_b"]
    q, k_l, v_l = np.split(qkv, 3, -1)
    qh = q.reshape(S, H, DH).transpose(1, 0, 2)
    kh = past_k[0]
    vh = past_v[0]
    sc = qh @ kh.transpose(0, 2, 1) / np.sqrt(DH)
    i = np.arange(S)[:, None]
    j = np.arange(S)[None, :]
    sc = np.where((j <= i)[None], sc, -np.inf)
    p = np.exp(sc - sc.max(-1, keepdims=True))
    p /= p.sum(-1, keepdims=True)
    at = (p @ vh).transpose(1, 0, 2).reshape(S, D)
    out_ref = at @ inputs["c_proj_w"] + inputs["c_proj_b"]

    def relerr(a, b):
        return np.abs(a - b).max() / max(np.abs(b).max(), 1e-9)

    print("out relerr:", relerr(out_dev, out_ref))
    print("k relerr:", relerr(kv_dev[:D].reshape(H, DH, S).transpose(0, 2, 1),
                              k_l.reshape(S, H, DH).transpose(1, 0, 2)))
    print("v relerr:", relerr(kv_dev[D:].reshape(H, DH, S).transpose(0, 2, 1),
                              v_l.reshape(S, H, DH).transpose(1, 0, 2)))
